# revision 42
# baseline (speedup 1.0000x reference)
"""AnyVariateAttention Trainium2 kernel (8 NeuronCores, SPMD, no collectives).

Problem: B=4, S=2048, D=512, H=8 attention with RoPE and a per-head
same-variate bias (u_same where variate_ids match, u_cross elsewhere),
softmax over keys, output projection.

Sharding: core c = 2*b + hf handles batch b and query-half hf (1024
queries, all 8 heads).  Attention rows are independent over queries, so
every core produces a disjoint slice of the output -- no collective.
To keep the SPMD graph identical across cores, x / ids / mask / key-side
rope tables are ROLLED host-side by the core's query offset (softmax and
PV are invariant to a permutation of the key axis), so the core's own
queries always sit in columns [0, 1024).

Algebraic folds (all exact w.r.t. the reference):
- variate bias: same[i,j] = onehot(id_i) . onehot(id_j) since ids in [0,8);
  scores+bias = [q*scale, oh_i, 1] . [k, du_h*oh_j, maskbias_j] with
  du_h = u_same[h]-u_cross[h]; the u_cross[h] term is uniform over j and
  cancels in softmax.  K extends 64 -> 73, free on the PE.
- mask: (1-mask_j)*-1e9 is the 73rd column (maskbias_j).
- softmax denominator: a ones column appended to V makes row 64 of the
  PV accumulation equal sum_j exp(s_ij); no standalone reduction needed.
- RoPE: rot(q) = q*C + swap_pairs(q)*S; swap_pairs is a DVE
  stream_shuffle (partition pair swap within 32-lane blocks).  The
  1/sqrt(64) scale is pre-multiplied into the q-side tables.
- projection biases: bq/bk fold into the psum->bf16 staging activation
  (per-partition bias on the Act engine); bv folds into the output bias
  (bo + Wo@bv) because softmax rows sum to 1.

Perf notes (all from HW traces):
- every dma_start costs ~0.6us of serial Sync-engine dispatch, so inputs
  are host-packed into 8 big DMAs and the rope chain uses zero DMAs.
- all heavy inputs are pre-cast to bf16 on the host: no on-device
  staging casts, halved input DMA bytes.  (fp8e4 inputs were tried for
  the projections: 2x fewer PE cycles via DoubleRow, but the D=512
  contraction amplifies e4m3 quantization to ~9% noise on q/k --
  rel err 6e-2, rejected.  fp8e5 probs for DoubleRow PV likewise.)
- phase C is exp-throughput-bound (16.8M score elements through the
  1.2GHz Act LUT = ~125us floor), so ~5/16 of each head's key-tiles
  are exp'd on the otherwise-idle DVE with a one-op Schraudolph
  bit-trick (affine fp32->int16 write aliased as bf16; +-4.2% noise
  that cancels in softmax up to a geometric-mean factor).  DVE's
  mandatory pipe-flush DRAIN caps it at ~2.1us/tile, hence the
  minority share.
- scores/PV run at [128,512] granularity on a 4-deep PSUM ring: the
  fine slots absorb the slower DVE tiles without stalling the PE
  (a PE idle gap > 3.4us triggers a HAM down-clock to K=4/8 that
  costs far more than the gap itself).
- per-head softmax denominators: ones-column in V accumulates the
  row-sum in PSUM row 64; the reciprocal+broadcast chain (Act copy,
  reshape DMA to [128,8], DVE recip, DRAM-bounce broadcast) is
  DEFERRED into the NEXT head's tile loop so its DMA latency never
  parks at the front of the DVE queue blocking exps.
- phase D: et<3 partial matmuls of 6 output tiles are emitted before
  the last head's chain resolves (2 of them on the po PSUM ring whose
  slot frees a head early), et3 split into two K=64 matmuls so the
  head-6 half runs during head 7's normalize; outputs ship as bf16 in
  2 batched DMAs (host upcasts).
"""

import sys
import types

import numpy as np
import ml_dtypes

# ---------------------------------------------------------------------------
# Environment patches (kernel.py must be self-contained).
# ---------------------------------------------------------------------------


def _install_patches():
    if 'antenv.axon_hooks' not in sys.modules:
        try:
            sys.path.insert(0, '/root/.axon_site/trn_agent_boot')
            import trn_boot
            hook = trn_boot._ntff_profile_via_ctypes('/opt/axon/libaxon_pjrt.so')
            mod = types.ModuleType('antenv.axon_hooks')
            mod.get_axon_ntff_profile_hook = lambda: hook
            mod.set_axon_ntff_profile_hook = lambda h: None
            sys.modules['antenv.axon_hooks'] = mod
        except Exception:
            pass

    # Walrus in this image accepts only one sync-wait on a CTRL (Drain)
    # instruction; TileContext's exit drain can carry several.  Spill the
    # extras onto following sync-engine nops (still before the all-engine
    # barrier, so semantics are unchanged).
    import concourse.tile as tile
    import concourse.mybir as mybir
    from concourse.vector_clock import ScopedClock

    if getattr(tile.TileContext, '_drain_patched', False):
        return

    def _drain_and_barrier(self, tick_clock, wait_clock):
        nc = self.nc
        drain_inst = nc.sync.drain()
        wait_clock.add_sem_waits(
            drain_inst.ins, ScopedClock({None: tick_clock.global_clock})
        )
        si = drain_inst.ins.sync_info
        waits = list(si.on_wait)
        if len(waits) > 1:
            si.on_wait = waits[:1]
            for w in waits[1:]:
                nop = nc.sync.nop()
                nop.ins.sync_info = mybir.SyncInfo(on_wait=[w], on_update=[])

        nc.all_engine_barrier()
        assert self.sems is not None
        popped = nc._tile_sem_poison_stack.pop()
        assert popped is self._sem_poison
        nc.clear_and_free_semaphores(list(self.sems.allocated().values()))
        nc.all_engine_barrier()

    tile.TileContext._drain_and_barrier = _drain_and_barrier
    tile.TileContext._drain_patched = True


_install_patches()

import concourse.bass as bass
import concourse.mybir as mybir
import concourse.tile as tile
from concourse.bass_utils import run_bass_kernel_spmd

# ---------------------------------------------------------------------------
# Problem constants (hardcoded per the spec).
# ---------------------------------------------------------------------------
B, S, D = 4, 2048, 512
H, HD = 8, 64
SI = S // 2      # queries per core
KE = HD + 8 + 1  # 73: extended contraction dim for scores
N_CORES = 8
ROPE_BASE = 10000.0
F32 = mybir.dt.float32
BF16 = mybir.dt.bfloat16
BF16NP = ml_dtypes.bfloat16

NKT = D // 128       # 4 k-tiles over model dim
NET = D // 128       # 4 e-tiles over projection dims (all 8 heads)
NST = S // 128       # 16 key tiles
VW = HD + 1          # 65: v plus ones column

PAIR_SWAP_MASK = [i ^ 1 for i in range(32)]

# Schraudolph-style exp on the DVE: bf16 bits of exp(x) ~= x*(128*log2 e)
# + (127*128 + C).  C=-7.3 centers the mantissa-linear approximation
# (+-4.2% multiplicative noise, geometric-mean exact); the uniform
# factor cancels in the softmax normalization.  Inputs are raw scores
# (|s| <~ 8), so the int16 result stays in [14700, 17800] -- no
# overflow, no sign issues.
SCHRAUD_SCALE = 128.0 * 1.4426950408889634
SCHRAUD_BIAS = 16248.7
E5 = mybir.dt.float8e5
E4 = mybir.dt.float8e4


def _rope_tables():
    inv_freq = 1.0 / (ROPE_BASE ** (np.arange(0, HD, 2, dtype=np.float64) / HD))
    t = np.arange(S, dtype=np.float64)
    freqs = np.outer(t, inv_freq)                  # (S, 32)
    emb = np.concatenate([freqs, freqs], axis=-1)  # (S, 64)
    ch = np.cos(emb)[:, ::2]                       # (S, 32)
    sh = np.sin(emb)[:, ::2]                       # (S, 32)
    C = np.empty((S, HD), dtype=np.float64)
    Sg = np.empty((S, HD), dtype=np.float64)
    C[:, 0::2] = ch
    C[:, 1::2] = ch
    Sg[:, 0::2] = -sh
    Sg[:, 1::2] = sh
    # transposed (64, S), tiled over the two heads of an e-tile -> (128, S)
    CT = np.tile(C.T, (2, 1))
    ST = np.tile(Sg.T, (2, 1))
    return CT, ST


def _free_bcast_ap(src, n):
    """Broadcast a DRAM source along a new (size n, stride 0) free dim
    inserted after the partition dim."""
    ap = [list(p) for p in src.ap]
    return bass.AP(tensor=src.tensor, offset=src.offset,
                   ap=[ap[0], [0, n]] + ap[1:])


def _bcast_ap(src, nparts):
    return bass.AP(tensor=src.tensor, offset=src.offset,
                   ap=[[0, nparts]] + [list(p) for p in src.ap][1:])


def build_graph():
    nc = bass.Bass(num_devices=N_CORES)

    # host-packed parameters (few, large DMAs: each dma_start costs ~0.6us
    # of serial Sync-engine dispatch)
    # x and the q/k/v projection weights ride in fp8e4 (weights host-scaled
    # by 8x so ~N(0, 1/512) values clear the e4m3 subnormal floor; the
    # 1/8 is recovered for free via the staging activation's scale)
    xP = nc.declare_dram_parameter("xP", [128, NKT, S], BF16, isOutput=False)
    # layout [128, et, which(q/k), kt, 128] so head-pair et0's q AND k
    # weights arrive in one small leading chunk
    wqkP = nc.declare_dram_parameter("wqk", [128, 2 * NKT * D], BF16,
                                     isOutput=False)
    wvP = nc.declare_dram_parameter("wv", [128, NKT * D], BF16, isOutput=False)
    woP = nc.declare_dram_parameter("wo", [128, NKT * D], BF16,
                                    isOutput=False)
    tabP = nc.declare_dram_parameter("tab", [128, 2 * SI + 2 * S], BF16,
                                     isOutput=False)
    qextP = nc.declare_dram_parameter("qext", [9, SI], BF16, isOutput=False)
    kextP = nc.declare_dram_parameter("kext", [9, H * S], BF16, isOutput=False)
    cstP = nc.declare_dram_parameter("cst", [128, 12], F32, isOutput=False)
    out_ext = nc.declare_dram_parameter("out", [D, SI], BF16, isOutput=True)

    rec_dram = nc.dram_tensor("rec_dram", [1, SI], F32)

    with tile.TileContext(nc) as tc:
        with tc.tile_pool(name="persist", bufs=1) as pp:
            # persistent tensors
            xT_bf = pp.tile([128, NKT, S], BF16, tag="xT_bf")
            wqk_bf = pp.tile([128, NET, 2, NKT, 128], BF16, tag="wqk")
            wv_sb = pp.tile([128, NKT, D], BF16, tag="wv")
            wo_sb = pp.tile([128, NKT, D], BF16, tag="wo")
            tab_bf = pp.tile([128, 2 * SI + 2 * S], BF16, tag="tab")
            cst_sb = pp.tile([128, 12], F32, tag="cst")
            qeF = pp.tile([KE, H, SI], BF16, tag="qeF")
            keF = pp.tile([KE, H, S], BF16, tag="keF")
            v_bf = pp.tile([128, NST, H, VW], BF16, tag="v_bf")
            o_bf = pp.tile([128, NET, 2, 512], BF16, tag="o_bf")

            cq = tab_bf[:, 0:SI]
            sq = tab_bf[:, SI:2 * SI]
            ck = tab_bf[:, 2 * SI:2 * SI + S]
            sk = tab_bf[:, 2 * SI + S:2 * SI + 2 * S]
            pb_col = cst_sb[:, 0:8]
            bo_sb = cst_sb[:, 8:12]

            # ---------------- input loads (7 packed DMAs) ------------------
            nc.sync.dma_start(out=wqk_bf[:, 0, :, :, :],
                              in_=wqkP[:, 0:2 * NKT * 128])
            nc.sync.dma_start(out=xT_bf[:, :, 0:512], in_=xP[:, :, 0:512])
            nc.sync.dma_start(out=xT_bf[:, :, 512:1024],
                              in_=xP[:, :, 512:1024])
            nc.sync.dma_start(out=wqk_bf[:, 1:NET, :, :, :],
                              in_=wqkP[:, 2 * NKT * 128:2 * NKT * D])
            nc.sync.dma_start(out=xT_bf[:, :, 1024:2048],
                              in_=xP[:, :, 1024:2048])
            nc.sync.dma_start(out=wv_sb, in_=wvP[:])
            nc.sync.dma_start(out=wo_sb, in_=woP[:])
            nc.sync.dma_start(out=tab_bf, in_=tabP[:])
            nc.sync.dma_start(out=cst_sb, in_=cstP[:])
            # extension rows: qext broadcast over heads, kext per head
            nc.sync.dma_start(out=qeF[HD:KE, :, :],
                              in_=_free_bcast_ap(qextP[:], H))
            nc.sync.dma_start(out=keF[HD:KE, :, :], in_=kextP[:])
            nc.vector.memset(v_bf[:, :, :, HD:VW], 1.0)

            # ------------- phase B: projections + rope, V interleaved ------
            with (
                tc.tile_pool(name="rope", bufs=1) as rp,
                tc.tile_pool(name="psB", bufs=1, space="PSUM") as psB,
            ):
                def stage_proj(p0, which, et, tg):
                    # psum -> bf16 SBUF stage with fused bias add, on the
                    # (otherwise idle) Act engine; frees the PSUM ring fast
                    # and makes every rope DVE op all-bf16 (2x mode).
                    # scale=1/8 undoes the host-side 8x weight scaling.
                    bcol = which * 4 + et
                    stage = rp.tile([128, 1024], BF16, tag="pstage",
                                    bufs=6, name=f"st_{tg}")
                    nc.scalar.activation(stage, p0[:, :],
                                         mybir.ActivationFunctionType.Identity,
                                         bias=pb_col[:, bcol:bcol + 1])
                    return stage

                def rope_tile(stage, which, et, sc):
                    # stage: [128, 1024] bf16, 2 heads of q or k proj.
                    # rot = stage*C + pair_swap(stage)*S_swapped; the pair
                    # swap is 2 small SBUF DMAs, the muls run on DVE (2x
                    # bf16 mode) and the final adds on the idle GpSimd,
                    # writing per head directly into the fused qeF/keF.
                    ctab = (cq, ck)[which]
                    stab = (sq, sk)[which]
                    dst = (qeF, keF)[which]
                    s0 = sc * 1024
                    tg = f"{et}_{which}_{sc}"
                    t1 = rp.tile([128, 1024], BF16, tag="ropet1",
                                 bufs=3, name=f"t1_{tg}")
                    nc.vector.tensor_mul(t1, stage[:, :], ctab[:, s0:s0 + 1024])
                    t2 = rp.tile([128, 1024], BF16, tag="ropet2",
                                 bufs=3, name=f"t2_{tg}")
                    nc.vector.tensor_mul(t2, stage[:, :], stab[:, s0:s0 + 1024])
                    t2s = rp.tile([128, 1024], BF16, tag="ropet2s",
                                  bufs=3, name=f"t2s_{tg}")
                    nc.sync.dma_start(out=t2s[0:128:2, :], in_=t2[1:128:2, :])
                    nc.sync.dma_start(out=t2s[1:128:2, :], in_=t2[0:128:2, :])
                    nc.vector.tensor_add(
                        dst[0:HD, et * 2, s0:s0 + 1024],
                        t1[0:64, :], t2s[0:64, :])
                    nc.gpsimd.tensor_add(
                        dst[0:HD, et * 2 + 1, s0:s0 + 1024],
                        t1[64:128, :], t2s[64:128, :])

                DR = mybir.MatmulPerfMode.DoubleRow
                for et in range(NET):
                    e0 = et * 128
                    # q projection for head pair et: [128, 1024]; fp8
                    # DoubleRow over kt PAIRS (contraction 256 per MM)
                    p0q = psB.tile([128, 1024], F32, tag="p0", bufs=3,
                                   name=f"p0q{et}")
                    for half in range(2):
                        o0 = half * 512
                        for kt in range(NKT):
                            nc.tensor.matmul(
                                p0q[:, o0:o0 + 512],
                                wqk_bf[:, et, 0, kt, :],
                                xT_bf[:, kt, o0:o0 + 512],
                                start=kt == 0, stop=kt == NKT - 1)
                    sq_st = stage_proj(p0q, 0, et, f"q{et}")
                    # k projection for head pair et: 2x [128, 1024]
                    k_st = []
                    for sc in range(2):
                        pk = psB.tile([128, 1024], F32, tag="p0", bufs=3,
                                      name=f"p0k{et}_{sc}")
                        for half in range(2):
                            hs = sc * 1024 + half * 512
                            o0 = half * 512
                            for kt in range(NKT):
                                nc.tensor.matmul(
                                    pk[:, o0:o0 + 512],
                                    wqk_bf[:, et, 1, kt, :],
                                    xT_bf[:, kt, hs:hs + 512],
                                    start=kt == 0, stop=kt == NKT - 1)
                        k_st.append(stage_proj(pk, 1, et, f"k{et}_{sc}"))
                    rope_tile(sq_st, 0, et, 0)
                    rope_tile(k_st[0], 1, et, 0)
                    rope_tile(k_st[1], 1, et, 1)
                    # v projection for key tiles 4et .. 4et+3
                    for st in range(4 * et, 4 * et + 4):
                        pv = psB.tile([128, 512], F32, tag="pv", bufs=2,
                                      name=f"pv{st}")
                        for kt in range(NKT):
                            nc.tensor.matmul(
                                pv,
                                xT_bf[:, kt, st * 128:(st + 1) * 128],
                                wv_sb[:, kt, :],
                                start=kt == 0, stop=kt == NKT - 1)
                        nc.scalar.activation(
                            v_bf[:, st, :, 0:HD],
                            pv.rearrange("p (h d) -> p h d", h=H),
                            mybir.ActivationFunctionType.Copy)

            # ---------------- phase C: attention per head -------------------
            with (
                tc.tile_pool(name="psS", bufs=1, space="PSUM") as psS,
                tc.tile_pool(name="psO", bufs=1, space="PSUM") as psO,
            ):
                # Normalize chain of head h-1 is EMITTED inside head h's
                # jt loop so its DMA-latency waits never sit at the front
                # of the DVE queue blocking the next exp (which stalls the
                # PE and triggers a HAM rethrottle).  Stages: reciprocal
                # (DVE, after jt2), DRAM-bounce broadcast (Sync, async),
                # final multiply (DVE, after jt9 -- DMA long done).
                norm_state = {}

                def norm_start(h, po):
                    # denominator row = row 64 of po (copied out on Act --
                    # DMA cannot read PSUM), then a reshaping DMA
                    # [1,1024] -> [128,8]: DVE reciprocal is ~6.4
                    # cycles/elem PER LANE, so the wide row must be spread
                    # across partitions.
                    den_row = pp.tile([1, 2, 512], F32, tag="den_row",
                                      bufs=2, name=f"dr{h}")
                    nc.scalar.activation(den_row, po[HD:VW, :, :],
                                         mybir.ActivationFunctionType.Copy)
                    rec_mat = pp.tile([128, 8], F32, tag="rec_mat", bufs=2,
                                      name=f"rm{h}")
                    nc.sync.dma_start(out=rec_mat, in_=den_row[:, :, :])
                    norm_state[h] = (po, rec_mat)

                def norm_recip(h):
                    po, rec_mat = norm_state[h]
                    rinv = pp.tile([128, 8], F32, tag="rinv", bufs=2,
                                   name=f"ri{h}")
                    nc.vector.reciprocal(rinv, rec_mat[:, :])
                    nc.sync.dma_start(out=rec_dram[:], in_=rinv[:, :])
                    rec_bc = pp.tile([64, 2, 512], F32, tag="rec_bc",
                                     bufs=2, name=f"rb{h}")
                    nc.sync.dma_start(out=rec_bc,
                                      in_=_bcast_ap(rec_dram[:], 64))
                    norm_state[h] = (po, rec_bc)

                def norm_mul(h):
                    po, rec_bc = norm_state.pop(h)
                    et, hh = h // 2, h % 2
                    nc.vector.tensor_mul(o_bf[hh * 64:hh * 64 + 64, et, :, :],
                                         po[0:HD, :, :], rec_bc[:, :, :])

                for h in range(H):
                    # declared full-height so phase D can reuse the same
                    # ring slots (shape must match per tag); PV only
                    # writes/reads rows 0..64
                    po = psO.tile([128, 2, 512], F32, tag="po", bufs=2,
                                  name=f"po{h}")
                    for jt in range(NST):
                        # DVE takes ~1/4 of the tiles: its effective exp
                        # throughput is only ~2.9us/tile (op + mandatory
                        # pipe-flush DRAIN), so a 50/50 split makes DVE
                        # the pacer.  h0/h1 stay mostly on Act while the
                        # rope tail drains off the Vector queue.
                        if h >= 2:
                            use_dve = jt in (0, 3, 6, 9, 12)
                        else:
                            use_dve = h == 1 and jt in (9, 13)
                        # per-(jt,q2) granularity: [128,512] PSUM slots in
                        # a 4-deep ring absorb the slower DVE tiles
                        # without stalling the score matmuls.
                        for q2 in range(2):
                            ps = psS.tile([128, 512], F32, tag="ps",
                                          bufs=4, name=f"ps{h}_{jt}_{q2}")
                            nc.tensor.matmul(
                                ps,
                                keF[:, h, jt * 128:(jt + 1) * 128],
                                qeF[:, h, q2 * 512:(q2 + 1) * 512],
                                start=True, stop=True)
                            probs = pp.tile([128, 512], BF16, tag="probs",
                                            bufs=8, name=f"pr{h}_{jt}_{q2}")
                            if use_dve:
                                nc.vector.tensor_scalar(
                                    out=probs[:, :].bitcast(mybir.dt.int16),
                                    in0=ps[:, :],
                                    scalar1=SCHRAUD_SCALE,
                                    scalar2=SCHRAUD_BIAS,
                                    op0=mybir.AluOpType.mult,
                                    op1=mybir.AluOpType.add)
                            else:
                                nc.scalar.activation(
                                    probs, ps[:, :],
                                    mybir.ActivationFunctionType.Exp)
                            nc.tensor.matmul(po[0:VW, q2, :],
                                             v_bf[:, jt, h, :],
                                             probs[:, :],
                                             start=jt == 0,
                                             stop=jt == NST - 1)
                        if h > 0 and jt == 2:
                            norm_recip(h - 1)
                        if h > 0 and jt == 9:
                            norm_mul(h - 1)
                    norm_start(h, po)
                # last head: chain runs exposed; keep it as short as it gets
                norm_recip(H - 1)
                norm_mul(H - 1)

                # -------------- phase D: output projection ------------------
                # reuses the psS "ps" ring (2x [128,2,512] = 4 pout units) so
                # there is no pool-WAR wait on the last head's po; the et<3
                # partial matmuls of the first 4 output tiles run while the
                # last head's normalize chain drains.
                outf_all = pp.tile([128, 4, 2, 512], BF16, tag="outf_all")

                def d_tile(ts, ets, finish, split_et3=False):
                    # ts: list of (pd_ap, ft, sc).  split_et3 runs et3 as
                    # two K=64 matmuls so the head-6 half can run while
                    # head 7's normalize chain drains.
                    for pd, ft, sc in ts:
                        for et in ets:
                            if et == 3 and split_et3:
                                continue
                            nc.tensor.matmul(
                                pd,
                                wo_sb[:, et, ft * 128:(ft + 1) * 128],
                                o_bf[:, et, sc, :],
                                start=et == 0, stop=et == 3)
                    if split_et3 and 3 in ets:
                        for p0 in (0, 64):
                            for pd, ft, sc in ts:
                                nc.tensor.matmul(
                                    pd,
                                    wo_sb[p0:p0 + 64, 3, ft * 128:(ft + 1) * 128],
                                    o_bf[p0:p0 + 64, 3, sc, :],
                                    start=False, stop=p0 == 64)
                    if not finish:
                        return
                    for pd, ft, sc in ts:
                        nc.vector.tensor_scalar_add(outf_all[:, ft, sc, :],
                                                    pd,
                                                    bo_sb[:, ft:ft + 1])

                # grp0 on the (now [128,512] x4) ps ring; grp1 rides the
                # po ring: pd_c's slot (head 6's) frees at norm_mul(6), so
                # its et<3 partials run during head 7's exposed normalize
                # chain, while pd_d (head 7's slot) serializes behind
                # norm_mul(7).
                grp0 = []
                for j in range(4):
                    pdj = psS.tile([128, 512], F32, tag="ps", bufs=4,
                                   name=f"pd{j}")
                    grp0.append((pdj[:, :], j // 2, j % 2))
                pd_c = psO.tile([128, 2, 512], F32, tag="po", bufs=2,
                                name="pd_c")
                pd_d = psO.tile([128, 2, 512], F32, tag="po", bufs=2,
                                name="pd_d")
                grp1 = [(pd_c[:, 0, :], 2, 0), (pd_c[:, 1, :], 2, 1),
                        (pd_d[:, 0, :], 3, 0), (pd_d[:, 1, :], 3, 1)]
                d_tile(grp0, range(3), finish=False)   # partials, no h7 dep
                d_tile(grp1[0:2], range(3), finish=False)
                d_tile(grp0, [3], finish=True, split_et3=True)
                d_tile(grp1[0:2], [3], finish=False, split_et3=True)
                d_tile(grp1[2:4], range(4), finish=False)
                # out rows 0..255 ship while grp1 finishes
                nc.sync.dma_start(
                    out=out_ext[:].rearrange("(f p) (s c) -> p f s c",
                                             f=4, s=2)[:, 0:2, :, :],
                    in_=outf_all[:, 0:2, :, :])
                for pd, ft, sc in grp1:
                    nc.vector.tensor_scalar_add(outf_all[:, ft, sc, :],
                                                pd,
                                                bo_sb[:, ft:ft + 1])
                nc.sync.dma_start(
                    out=out_ext[:].rearrange("(f p) (s c) -> p f s c",
                                             f=4, s=2)[:, 2:4, :, :],
                    in_=outf_all[:, 2:4, :, :])

    _spill_sync_waits(nc)
    return nc


def _spill_sync_waits(nc, max_waits=1):
    """Walrus in this image allows very few sync-wait commands per
    instruction.  Hoist extras onto same-engine nops placed just before the
    instruction (same blocking semantics on the engine's sequencer)."""
    for bb in nc.cur_f.blocks:
        new = []
        changed = False
        for inst in bb.instructions:
            si = inst.sync_info
            waits = list(si.on_wait) if si is not None else []
            if len(waits) > max_waits:
                for w in waits[:-max_waits]:
                    nop = mybir.InstNoOp(name=f"spillw-{nc.next_id()}",
                                         engine=inst.engine, ins=[], outs=[])
                    nop.sync_info = mybir.SyncInfo(on_wait=[w], on_update=[])
                    new.append(nop)
                si.on_wait = waits[-max_waits:]
                changed = True
            new.append(inst)
        if changed:
            bb.instructions = new


_GRAPH = None


def _get_graph():
    global _GRAPH
    if _GRAPH is None:
        _GRAPH = build_graph()
    return _GRAPH


def make_in_maps(x, variate_ids, mask, Wq, bq, Wk, bk, Wv, bv, Wo, bo,
                 u_same, u_cross):
    CT, ST = _rope_tables()
    scale = 1.0 / np.sqrt(HD)

    x = np.asarray(x, dtype=np.float32)
    variate_ids = np.asarray(variate_ids)
    mask = np.asarray(mask, dtype=np.float32)
    Wq, Wk, Wv, Wo = (np.asarray(a, dtype=np.float32) for a in (Wq, Wk, Wv, Wo))
    bq, bk, bv, bo = (np.asarray(a, dtype=np.float32) for a in (bq, bk, bv, bo))
    duv = (np.asarray(u_same) - np.asarray(u_cross)).astype(np.float32)

    def bf(a):
        return np.ascontiguousarray(np.asarray(a, np.float32).astype(BF16NP))

    def f8(a):
        # TRN fp8e4 (e4m3); values here stay well under the 240 clip
        return np.ascontiguousarray(
            np.asarray(a, np.float32).astype(ml_dtypes.float8_e4m3fn))

    def pack_w(WT):
        # [D, D] -> [128, NKT, D]: row r holds WT[kt*128+r, :] for each kt
        return WT.reshape(NKT, 128, D).transpose(1, 0, 2)

    # wqk layout [128, et, which, kt, 128]: et0's q+k weights lead
    wq_t = pack_w(Wq.T).reshape(128, NKT, NET, 128)   # [r, kt, et, c]
    wk_t = pack_w(Wk.T).reshape(128, NKT, NET, 128)
    # q/k/v weights ship as fp8e4 scaled by 8 (the staging activation
    # multiplies by 1/8); Wo stays bf16 (its products feed the output
    # directly, fp8 there costs accuracy with no phase-D win)
    wqk = bf(np.stack([wq_t, wk_t], axis=1)            # [r, which, kt, et, c]
             .transpose(0, 3, 1, 2, 4)                 # [r, et, which, kt, c]
             .reshape(128, 2 * NKT * D))
    wv = bf(pack_w(Wv.T).reshape(128, NKT * D))
    wo = bf(pack_w(Wo.T).reshape(128, NKT * D))

    in_maps = []
    for c in range(N_CORES):
        b, hf = c // 2, c % 2
        off = hf * SI

        def roll(a):
            return np.roll(a, -off, axis=-1)

        ids_r = roll(variate_ids[b].astype(np.int64))
        mask_r = roll(mask[b])
        # extension rows: qext = [onehot(id_q); ones], kext per head
        # = [du_h * onehot(id_k); -1e9*(1-mask_k)]
        oh_q = (ids_r[None, :SI] == np.arange(8)[:, None]).astype(np.float32)
        oh_k = (ids_r[None, :] == np.arange(8)[:, None]).astype(np.float32)
        qext = np.concatenate([oh_q, np.ones((1, SI), np.float32)], axis=0)
        maskbias = (-1e9 * (1.0 - mask_r)).reshape(1, S)
        # [9, H, S] flattened to [9, H*S]
        kext = np.stack(
            [np.concatenate([duv[h] * oh_k, maskbias], axis=0)
             for h in range(H)], axis=1).reshape(9, H * S)

        STs = ST[np.arange(128) ^ 1]  # row-pair-swapped S table
        tab = np.concatenate([
            CT[:, off:off + SI] * scale, STs[:, off:off + SI] * scale,
            roll(CT), roll(STs)], axis=1)
        cst = np.concatenate([
            np.concatenate([bq.reshape(4, 128).T, bk.reshape(4, 128).T],
                           axis=1),
            (bo + Wo @ bv).reshape(NKT, 128).T], axis=1).astype(np.float32)

        xr = roll(x[b].T)  # [D, S]
        in_maps.append({
            "xP": bf(xr.reshape(NKT, 128, S).transpose(1, 0, 2)),
            "wqk": wqk, "wv": wv, "wo": wo,
            "tab": bf(tab),
            "qext": bf(qext),
            "kext": bf(kext),
            "cst": np.ascontiguousarray(cst),
        })
    return in_maps


def run(inputs, trace=False):
    nc = _get_graph()
    in_maps = make_in_maps(**inputs)
    try:
        res = run_bass_kernel_spmd(nc, in_maps, list(range(N_CORES)),
                                   trace=trace)
    except Exception:
        # transient device faults (NRT_EXEC_UNIT_UNRECOVERABLE) have been
        # observed on a first execution and recover on retry
        import time
        time.sleep(2)
        res = run_bass_kernel_spmd(nc, in_maps, list(range(N_CORES)),
                                   trace=trace)
    out = np.empty((B, S, D), dtype=np.float32)
    for c in range(N_CORES):
        b, hf = c // 2, c % 2
        out[b, hf * SI:(hf + 1) * SI, :] = \
            np.asarray(res.results[c]["out"]).astype(np.float32).T
    return out, res


def kernel(**inputs) -> np.ndarray:
    out, _ = run(inputs, trace=False)
    return out

